# revision 26
# baseline (speedup 1.0000x reference)
"""Trainium2 Bass kernel for AttributionCentroidTracker.

Reference computation (B=512, V=32768, C=16):
    Wg[b, v]   = W_eff[b, v, labels[b]]
    attr[b, v] = |sparse_vector[b, v] * Wg[b, v]|
    sums[c, v] = segment_sum(attr, labels)       # [C, V]
    mean       = sums / max(counts, 1)
    out[c]     = centroids[c]                     if counts[c] == 0
               = mean[c]                          if not initialized[c]
               = M*centroids[c] + (1-M)*mean[c]   otherwise

Device strategy (8 cores, sharded along V — per-class sums are complete
locally per V-slice, so no cross-core reduction is needed):
  - b (512) lives on the 128 SBUF partitions in 4 groups of 128.
  - W streams as PLAIN f32 over HWDGE (measured 395 GB/s/core vs only
    310 GB/s for the SWDGE cast-DMA path — the in-DMA dtype conversion
    is the slower wall).  ScalarE then does |W| WITH the f32->bf16 cast
    in one Abs pass, writing the bf16 result over the front half of the
    same tile (writer address trails reader address, so in-place is
    safe).
  - VectorE multiplies by |sv| broadcast along c (stride-0, 1x mode).
  - Segment-sum on TensorE with plain one-hot lhsT [128,16] per batch
    group and contiguous rhs [128,512] (v32*c16 layout), accumulating
    over the 4 groups into 4 PSUM banks per tile.  psum[c', (v,c)]
    holds per-class sums of ALL 16 channels; only the diagonal c'==c is
    wanted.
  - Banks are evacuated scaled by a_c into a bf16 staging tile (split
    between ScalarE and VectorE); every TBATCH tiles the diagonal is
    pulled out with 16 accumulating selection matmuls E_cc^T @
    stage[:, :, :, c] and added into the f32 accumulator out_sb, which
    is pre-loaded with b_c*centroids (host-computed).
  - a/b host math: a = (init ? (1-M)/n : 1/n) if present else 0,
    b = (init ? M : 0) if present else 1.
"""

import os
import sys

import numpy as np

if "/opt/trn_rl_repo" not in sys.path:
    sys.path.insert(0, "/opt/trn_rl_repo")

B, V, C = 512, 32768, 16
NCORES = 8
VSH = V // NCORES            # 4096 columns of V per core
P = 128                      # SBUF partitions
BG = B // P                  # 4 batch groups
VC = 64                      # v-chunk per tile
NVC = VSH // VC              # 64 tiles per core
NSUB = 2                     # psum banks per tile (32 v each)
VSUB = VC // NSUB            # 32
TBATCH = 8                   # tiles per extraction batch
NQ = 8                       # sv chunk-loads (eighths)
VQ = VSH // NQ               # 512
STEPS_PER_EPOCH = 1000
MOMENTUM = 1.0 - 2.0 / (STEPS_PER_EPOCH + 1)

_CACHE = {}

last_exec_time_ns = None
last_results = None


def _build_nc():
    import concourse.bacc as bacc
    import concourse.tile as tile
    from concourse import mybir

    f32 = mybir.dt.float32
    bf16 = mybir.dt.bfloat16
    Copy = mybir.ActivationFunctionType.Copy
    Abs = mybir.ActivationFunctionType.Abs
    nc = bacc.Bacc("TRN2", target_bir_lowering=False, debug=False)

    w = nc.dram_tensor("w", [B, VSH, C], f32, kind="ExternalInput")
    sv = nc.dram_tensor("sv", [B, VSH], f32, kind="ExternalInput")
    oh = nc.dram_tensor("oh", [P, BG * C], bf16, kind="ExternalInput")
    sel = nc.dram_tensor("sel", [C, C * C], bf16, kind="ExternalInput")
    centb = nc.dram_tensor("centb", [C, VSH], f32, kind="ExternalInput")
    avec = nc.dram_tensor("avec", [C, 1], f32, kind="ExternalInput")
    out = nc.dram_tensor("out", [C, VSH], f32, kind="ExternalOutput")

    # b = g*128 + p  ->  partition p, group g
    w_r = w.ap().rearrange("(g p) v c -> p g v c", p=P)      # [128, 4, VSH, 16]
    sv_r = sv.ap().rearrange("(g p) v -> p g v", p=P)        # [128, 4, VSH]

    with tile.TileContext(nc) as tc:
        with (
            tc.tile_pool(name="const", bufs=1) as cpool,
            tc.tile_pool(name="wp", bufs=5) as wpool,
            tc.tile_pool(name="wbp", bufs=4) as wbpool,
            tc.tile_pool(name="svq", bufs=2) as qpool,
            tc.tile_pool(name="stg", bufs=2) as spool,
            tc.tile_pool(name="psum", bufs=8, space="PSUM") as ppool,
        ):
            # |sv| quarters as bf16 via SWDGE cast-DMA (small; the big W
            # stream stays on the faster plain-HWDGE path), abs in place.
            def issue_sv_quarter(q):
                qsl = slice(q * VQ, (q + 1) * VQ)
                svq = qpool.tile([P, BG * VQ], dtype=bf16, tag="svq")
                svq3 = svq[:].rearrange("p (g v) -> p g v", g=BG)
                nc.gpsimd.dma_start(out=svq3, in_=sv_r[:, :, qsl])
                qv = svq[:].bitcast(mybir.dt.int32)
                nc.vector.tensor_scalar(
                    out=qv,
                    in0=qv,
                    scalar1=0x7FFF7FFF,
                    scalar2=None,
                    op0=mybir.AluOpType.bitwise_and,
                )
                return svq3

            svq_cur = issue_sv_quarter(0)

            oh_sb = cpool.tile([P, BG * C], dtype=bf16)
            nc.sync.dma_start(out=oh_sb[:], in_=oh.ap())
            sel_sb = cpool.tile([C, C * C], dtype=bf16)
            nc.sync.dma_start(out=sel_sb[:], in_=sel.ap())
            avec_sb = cpool.tile([C, 1], dtype=f32)
            nc.sync.dma_start(out=avec_sb[:], in_=avec.ap())

            # accumulator pre-loaded with b_c * centroids
            out_sb = cpool.tile([C, VSH], dtype=f32)
            nc.sync.dma_start(out=out_sb[:], in_=centb.ap())

            def issue_w_dma(i):
                vlo = i * VC
                wt = wpool.tile([P, BG * VC * C], dtype=f32, tag="wt")
                wt4 = wt[:].rearrange("p (g v c) -> p g v c", g=BG, v=VC)
                nc.sync.dma_start(out=wt4, in_=w_r[:, :, vlo : vlo + VC, :])
                return wt

            PREFETCH = 3
            prefetched = {}
            for i in range(min(PREFETCH, NVC)):
                prefetched[i] = issue_w_dma(i)

            nsv = 1
            stage = None
            svq_next = None
            for i in range(NVC):
                vlo = i * VC
                ib = i % TBATCH

                if i + PREFETCH < NVC:
                    prefetched[i + PREFETCH] = issue_w_dma(i + PREFETCH)
                # next sv quarter two tiles ahead of first use
                if nsv < NQ and i == (nsv * NVC // NQ) - 2:
                    svq_next = issue_sv_quarter(nsv)
                if nsv < NQ and i == (nsv * NVC // NQ):
                    svq_cur = svq_next
                    nsv += 1

                wt = prefetched.pop(i)
                vq = vlo - (vlo // VQ) * VQ
                if ib == 0:
                    stage = spool.tile(
                        [C, TBATCH * VC * C], dtype=bf16, tag="stage"
                    )
                # stage layout is (c, k, v) so the diagonal matmuls below
                # read contiguous [16, 512] rhs slices per class
                nchunk = TBATCH * NSUB
                stg_ev = stage[:].rearrange(
                    "q (c k v) -> q k v c", c=C, k=nchunk, v=VSUB
                )

                # |W| with the f32->bf16 cast in one ScalarE pass into a
                # separate bf16 tile (frees the f32 slot for the DMA early)
                wb_t = wbpool.tile([P, BG * VC * C], dtype=bf16, tag="wb")
                wb = wb_t[:]
                nc.scalar.activation(wb, wt[:], Abs)
                # Y = |W| * |sv|  (|sv| broadcast along c, DVE 1x)
                wb4 = wb.rearrange("p (g v c) -> p g v c", g=BG, v=VC)
                in1 = (
                    svq_cur[:, :, vq : vq + VC]
                    .unsqueeze(3)
                    .broadcast_to([P, BG, VC, C])
                )
                nc.vector.tensor_tensor(
                    out=wb4, in0=wb4, in1=in1, op=mybir.AluOpType.mult
                )

                # segment-sum: ps[c', (v32, c)] += oh_g^T @ Y_g
                # (g outer so each one-hot block is loaded once into PE)
                pss = []
                for s in range(NSUB):
                    pss.append(
                        ppool.tile(
                            [C, VSUB * C],
                            dtype=mybir.dt.float32,
                            tag="ps",
                            name=f"ps{s}_{i}",
                        )
                    )
                for g in range(BG):
                    for s in range(NSUB):
                        off = g * (VC * C) + s * (VSUB * C)
                        nc.tensor.matmul(
                            out=pss[s][:],
                            lhsT=oh_sb[:, g * C : (g + 1) * C],
                            rhs=wb[:, off : off + VSUB * C],
                            start=(g == 0),
                            stop=(g == BG - 1),
                        )
                for s in range(NSUB):
                    # evacuate scaled by a_c into the staging tile in
                    # (c, k, v) order, alternating engines for balance
                    k = ib * NSUB + s
                    dst = stg_ev[:, k]
                    if s % 2 == 0:
                        nc.scalar.activation(
                            dst, pss[s][:], Copy, bias=0.0, scale=avec_sb[:]
                        )
                    else:
                        nc.vector.tensor_scalar(
                            out=dst,
                            in0=pss[s][:],
                            scalar1=avec_sb[:],
                            scalar2=None,
                            op0=mybir.AluOpType.mult,
                        )

                # extraction batch: diagonal via 16 accumulating selection
                # matmuls E_cc^T @ stage[:, c-block] (contiguous rhs)
                if ib == TBATCH - 1:
                    ps2 = ppool.tile(
                        [C, TBATCH * VC],
                        dtype=mybir.dt.float32,
                        tag="ps",
                        name=f"ps_diag_{i}",
                    )
                    for c in range(C):
                        nc.tensor.matmul(
                            out=ps2[:],
                            lhsT=sel_sb[:, c * C : (c + 1) * C],
                            rhs=stage[:, c * nchunk * VSUB : (c + 1) * nchunk * VSUB],
                            start=(c == 0),
                            stop=(c == C - 1),
                        )
                    ooff = (i - (TBATCH - 1)) * VC
                    nc.vector.tensor_tensor(
                        out=out_sb[:, ooff : ooff + TBATCH * VC],
                        in0=out_sb[:, ooff : ooff + TBATCH * VC],
                        in1=ps2[:],
                        op=mybir.AluOpType.add,
                    )

            nc.sync.dma_start(out=out.ap(), in_=out_sb[:])

    nc.finalize()
    return nc


def _get_nc():
    if "nc" not in _CACHE:
        _CACHE["nc"] = _build_nc()
    return _CACHE["nc"]


def kernel(sparse_vector, W_eff, labels, centroids, initialized):
    global last_exec_time_ns, last_results
    import ml_dtypes
    from concourse.bass_utils import run_bass_kernel_spmd

    sv = np.ascontiguousarray(np.asarray(sparse_vector, dtype=np.float32))
    w = np.asarray(W_eff, dtype=np.float32)
    lab = np.asarray(labels).astype(np.int64)
    cent = np.asarray(centroids, dtype=np.float32)
    init = np.asarray(initialized).astype(bool)

    # Host-side label-derived constants (tiny) — keep the program generic.
    ohm = lab[:, None] == np.arange(C)[None, :]          # [B, C] bool
    counts = ohm.sum(axis=0).astype(np.float64)          # [C]
    present = counts > 0
    safe = np.maximum(counts, 1.0)
    a = np.where(present, np.where(init, (1.0 - MOMENTUM) / safe, 1.0 / safe), 0.0)
    b = np.where(present, np.where(init, MOMENTUM, 0.0), 1.0)
    avec = a.astype(np.float32).reshape(C, 1)
    centb = (b[:, None] * cent.astype(np.float64)).astype(np.float32)  # [C, V]

    # Plain one-hot lhsT blocks: oh[p, g*C + c] = 1 iff labels[g*128+p]==c
    lab2 = lab.reshape(BG, P)                            # [g, p]
    oh = np.zeros((P, BG * C), np.float32)
    for g in range(BG):
        oh[np.arange(P), g * C + lab2[g]] = 1.0
    oh = oh.astype(ml_dtypes.bfloat16)

    # Diagonal-selection lhsT blocks: sel[p, c*C+m] = 1 iff p==c==m
    selm = np.zeros((C, C * C), np.float32)
    for c in range(C):
        selm[c, c * C + c] = 1.0
    selm = selm.astype(ml_dtypes.bfloat16)

    nc = _get_nc()
    in_maps = []
    for i in range(NCORES):
        s = i * VSH
        in_maps.append(
            {
                "w": np.ascontiguousarray(w[:, s : s + VSH, :]),
                "sv": np.ascontiguousarray(sv[:, s : s + VSH]),
                "oh": oh,
                "sel": selm,
                "centb": np.ascontiguousarray(centb[:, s : s + VSH]),
                "avec": avec,
            }
        )

    res = run_bass_kernel_spmd(nc, in_maps, core_ids=list(range(NCORES)))
    last_exec_time_ns = res.exec_time_ns
    last_results = res
    return np.concatenate([res.results[i]["out"] for i in range(NCORES)], axis=1)


# revision 30
# speedup vs baseline: 1.0104x; 1.0104x over previous
"""Trainium2 Bass kernel for AttributionCentroidTracker.

Reference computation (B=512, V=32768, C=16):
    Wg[b, v]   = W_eff[b, v, labels[b]]
    attr[b, v] = |sparse_vector[b, v] * Wg[b, v]|
    sums[c, v] = segment_sum(attr, labels)       # [C, V]
    mean       = sums / max(counts, 1)
    out[c]     = centroids[c]                     if counts[c] == 0
               = mean[c]                          if not initialized[c]
               = M*centroids[c] + (1-M)*mean[c]   otherwise

Device strategy (8 cores, sharded along V — per-class sums are complete
locally per V-slice, so no cross-core reduction is needed):
  - b (512) lives on the 128 SBUF partitions in 4 groups of 128.
  - W streams as PLAIN f32 over HWDGE (measured 395 GB/s/core vs only
    310 GB/s for the SWDGE cast-DMA path — the in-DMA dtype conversion
    is the slower wall).  ScalarE then does |W| WITH the f32->bf16 cast
    in one Abs pass, writing the bf16 result over the front half of the
    same tile (writer address trails reader address, so in-place is
    safe).
  - VectorE multiplies by |sv| broadcast along c (stride-0, 1x mode).
  - Segment-sum on TensorE with plain one-hot lhsT [128,16] per batch
    group and contiguous rhs [128,512] (v32*c16 layout), accumulating
    over the 4 groups into 4 PSUM banks per tile.  psum[c', (v,c)]
    holds per-class sums of ALL 16 channels; only the diagonal c'==c is
    wanted.
  - Banks are evacuated scaled by a_c into a bf16 staging tile (split
    between ScalarE and VectorE); every TBATCH tiles the diagonal is
    pulled out with 16 accumulating selection matmuls E_cc^T @
    stage[:, :, :, c] and added into the f32 accumulator out_sb, which
    is pre-loaded with b_c*centroids (host-computed).
  - a/b host math: a = (init ? (1-M)/n : 1/n) if present else 0,
    b = (init ? M : 0) if present else 1.
"""

import os
import sys

import numpy as np

if "/opt/trn_rl_repo" not in sys.path:
    sys.path.insert(0, "/opt/trn_rl_repo")

B, V, C = 512, 32768, 16
NCORES = 8
VSH = V // NCORES            # 4096 columns of V per core
P = 128                      # SBUF partitions
BG = B // P                  # 4 batch groups
VC = 128                     # v-chunk per tile
NVC = VSH // VC              # 32 tiles per core
NSUB = 4                     # psum banks per tile (32 v each)
VSUB = VC // NSUB            # 32
TBATCH = 4                   # tiles per extraction batch
NQ = 8                       # sv chunk-loads (eighths)
VQ = VSH // NQ               # 512
STEPS_PER_EPOCH = 1000
MOMENTUM = 1.0 - 2.0 / (STEPS_PER_EPOCH + 1)

_CACHE = {}

last_exec_time_ns = None
last_results = None


def _build_nc():
    import concourse.bacc as bacc
    import concourse.tile as tile
    from concourse import mybir

    f32 = mybir.dt.float32
    bf16 = mybir.dt.bfloat16
    Copy = mybir.ActivationFunctionType.Copy
    Abs = mybir.ActivationFunctionType.Abs
    nc = bacc.Bacc("TRN2", target_bir_lowering=False, debug=False)

    w = nc.dram_tensor("w", [B, VSH, C], f32, kind="ExternalInput")
    sv = nc.dram_tensor("sv", [B, VSH], f32, kind="ExternalInput")
    oh = nc.dram_tensor("oh", [P, BG * C], bf16, kind="ExternalInput")
    sel = nc.dram_tensor("sel", [C, C * C], bf16, kind="ExternalInput")
    centb = nc.dram_tensor("centb", [C, VSH], f32, kind="ExternalInput")
    avec = nc.dram_tensor("avec", [C, 1], f32, kind="ExternalInput")
    out = nc.dram_tensor("out", [C, VSH], f32, kind="ExternalOutput")

    # b = g*128 + p  ->  partition p, group g
    w_r = w.ap().rearrange("(g p) v c -> p g v c", p=P)      # [128, 4, VSH, 16]
    sv_r = sv.ap().rearrange("(g p) v -> p g v", p=P)        # [128, 4, VSH]

    with tile.TileContext(nc) as tc:
        with (
            tc.tile_pool(name="const", bufs=1) as cpool,
            tc.tile_pool(name="wp", bufs=6) as wpool,
            tc.tile_pool(name="wbp", bufs=4) as wbpool,
            tc.tile_pool(name="svq", bufs=2) as qpool,
            tc.tile_pool(name="stg", bufs=2) as spool,
            tc.tile_pool(name="psum", bufs=8, space="PSUM") as ppool,
        ):
            # |sv| quarters as bf16 via SWDGE cast-DMA (small; the big W
            # stream stays on the faster plain-HWDGE path), abs in place.
            def issue_sv_quarter(q):
                qsl = slice(q * VQ, (q + 1) * VQ)
                svq = qpool.tile([P, BG * VQ], dtype=bf16, tag="svq")
                svq3 = svq[:].rearrange("p (g v) -> p g v", g=BG)
                nc.gpsimd.dma_start(out=svq3, in_=sv_r[:, :, qsl])
                qv = svq[:].bitcast(mybir.dt.int32)
                nc.vector.tensor_scalar(
                    out=qv,
                    in0=qv,
                    scalar1=0x7FFF7FFF,
                    scalar2=None,
                    op0=mybir.AluOpType.bitwise_and,
                )
                return svq3

            svq_cur = issue_sv_quarter(0)

            oh_sb = cpool.tile([P, BG * C], dtype=bf16)
            nc.sync.dma_start(out=oh_sb[:], in_=oh.ap())
            sel_sb = cpool.tile([C, C * C], dtype=bf16)
            nc.sync.dma_start(out=sel_sb[:], in_=sel.ap())
            avec_sb = cpool.tile([C, 1], dtype=f32)
            nc.sync.dma_start(out=avec_sb[:], in_=avec.ap())

            # accumulator pre-loaded with b_c * centroids
            out_sb = cpool.tile([C, VSH], dtype=f32)
            nc.sync.dma_start(out=out_sb[:], in_=centb.ap())

            def issue_w_dma(i):
                # two g-pair units per v-chunk: finer pipeline granularity
                # at unchanged 8 KB contiguous DMA runs
                vlo = (i // 2) * VC
                u = i % 2
                wt = wpool.tile([P, 2 * VC * C], dtype=f32, tag="wt")
                wt4 = wt[:].rearrange("p (g v c) -> p g v c", g=2, v=VC)
                nc.sync.dma_start(
                    out=wt4, in_=w_r[:, 2 * u : 2 * u + 2, vlo : vlo + VC, :]
                )
                return wt

            PREFETCH = 6
            NUNIT = 2 * NVC
            prefetched = {}
            for j in range(min(PREFETCH, NUNIT)):
                prefetched[j] = issue_w_dma(j)

            nsv = 1
            stage = None
            svq_next = None
            for i in range(NVC):
                vlo = i * VC
                ib = i % TBATCH

                # next sv eighth two tiles ahead of first use
                if nsv < NQ and i == (nsv * NVC // NQ) - 2:
                    svq_next = issue_sv_quarter(nsv)
                if nsv < NQ and i == (nsv * NVC // NQ):
                    svq_cur = svq_next
                    nsv += 1

                vq = vlo - (vlo // VQ) * VQ
                if ib == 0:
                    stage = spool.tile(
                        [C, TBATCH * VC * C], dtype=bf16, tag="stage"
                    )
                # stage layout is (c, k, v) so the diagonal matmuls below
                # read contiguous [16, 512] rhs slices per class
                nchunk = TBATCH * NSUB
                stg_ev = stage[:].rearrange(
                    "q (c k v) -> q k v c", c=C, k=nchunk, v=VSUB
                )

                pss = []
                for s in range(NSUB):
                    pss.append(
                        ppool.tile(
                            [C, VSUB * C],
                            dtype=mybir.dt.float32,
                            tag="ps",
                            name=f"ps{s}_{i}",
                        )
                    )
                for u in range(2):
                    j = 2 * i + u
                    if j + PREFETCH < NUNIT:
                        prefetched[j + PREFETCH] = issue_w_dma(j + PREFETCH)
                    wt = prefetched.pop(j)
                    # |W| with the f32->bf16 cast in one ScalarE pass into
                    # a separate bf16 tile (frees the f32 slot early)
                    wb_t = wbpool.tile([P, 2 * VC * C], dtype=bf16, tag="wb")
                    wb = wb_t[:]
                    nc.scalar.activation(wb, wt[:], Abs)
                    # Y = |W| * |sv|  (|sv| broadcast along c, DVE 1x)
                    wb4 = wb.rearrange("p (g v c) -> p g v c", g=2, v=VC)
                    in1 = (
                        svq_cur[:, 2 * u : 2 * u + 2, vq : vq + VC]
                        .unsqueeze(3)
                        .broadcast_to([P, 2, VC, C])
                    )
                    nc.vector.tensor_tensor(
                        out=wb4, in0=wb4, in1=in1, op=mybir.AluOpType.mult
                    )
                    # segment-sum: ps[c', (v32, c)] += oh_g^T @ Y_g
                    for gu in range(2):
                        g = 2 * u + gu
                        for s in range(NSUB):
                            off = gu * (VC * C) + s * (VSUB * C)
                            nc.tensor.matmul(
                                out=pss[s][:],
                                lhsT=oh_sb[:, g * C : (g + 1) * C],
                                rhs=wb[:, off : off + VSUB * C],
                                start=(g == 0),
                                stop=(g == BG - 1),
                            )
                for s in range(NSUB):
                    # evacuate scaled by a_c into the staging tile in
                    # (c, k, v) order, alternating engines for balance
                    k = ib * NSUB + s
                    dst = stg_ev[:, k]
                    if s % 2 == 0:
                        nc.scalar.activation(
                            dst, pss[s][:], Copy, bias=0.0, scale=avec_sb[:]
                        )
                    else:
                        nc.vector.tensor_scalar(
                            out=dst,
                            in0=pss[s][:],
                            scalar1=avec_sb[:],
                            scalar2=None,
                            op0=mybir.AluOpType.mult,
                        )

                # extraction batch: diagonal via 16 accumulating selection
                # matmuls E_cc^T @ stage[:, c-block] (contiguous rhs)
                if ib == TBATCH - 1:
                    ps2 = ppool.tile(
                        [C, TBATCH * VC],
                        dtype=mybir.dt.float32,
                        tag="ps",
                        name=f"ps_diag_{i}",
                    )
                    for c in range(C):
                        nc.tensor.matmul(
                            out=ps2[:],
                            lhsT=sel_sb[:, c * C : (c + 1) * C],
                            rhs=stage[:, c * nchunk * VSUB : (c + 1) * nchunk * VSUB],
                            start=(c == 0),
                            stop=(c == C - 1),
                        )
                    ooff = (i - (TBATCH - 1)) * VC
                    nc.vector.tensor_tensor(
                        out=out_sb[:, ooff : ooff + TBATCH * VC],
                        in0=out_sb[:, ooff : ooff + TBATCH * VC],
                        in1=ps2[:],
                        op=mybir.AluOpType.add,
                    )

            nc.sync.dma_start(out=out.ap(), in_=out_sb[:])

    nc.finalize()
    return nc


def _get_nc():
    if "nc" not in _CACHE:
        _CACHE["nc"] = _build_nc()
    return _CACHE["nc"]


def kernel(sparse_vector, W_eff, labels, centroids, initialized):
    global last_exec_time_ns, last_results
    import ml_dtypes
    from concourse.bass_utils import run_bass_kernel_spmd

    sv = np.ascontiguousarray(np.asarray(sparse_vector, dtype=np.float32))
    w = np.asarray(W_eff, dtype=np.float32)
    lab = np.asarray(labels).astype(np.int64)
    cent = np.asarray(centroids, dtype=np.float32)
    init = np.asarray(initialized).astype(bool)

    # Host-side label-derived constants (tiny) — keep the program generic.
    ohm = lab[:, None] == np.arange(C)[None, :]          # [B, C] bool
    counts = ohm.sum(axis=0).astype(np.float64)          # [C]
    present = counts > 0
    safe = np.maximum(counts, 1.0)
    a = np.where(present, np.where(init, (1.0 - MOMENTUM) / safe, 1.0 / safe), 0.0)
    b = np.where(present, np.where(init, MOMENTUM, 0.0), 1.0)
    avec = a.astype(np.float32).reshape(C, 1)
    centb = (b[:, None] * cent.astype(np.float64)).astype(np.float32)  # [C, V]

    # Plain one-hot lhsT blocks: oh[p, g*C + c] = 1 iff labels[g*128+p]==c
    lab2 = lab.reshape(BG, P)                            # [g, p]
    oh = np.zeros((P, BG * C), np.float32)
    for g in range(BG):
        oh[np.arange(P), g * C + lab2[g]] = 1.0
    oh = oh.astype(ml_dtypes.bfloat16)

    # Diagonal-selection lhsT blocks: sel[p, c*C+m] = 1 iff p==c==m
    selm = np.zeros((C, C * C), np.float32)
    for c in range(C):
        selm[c, c * C + c] = 1.0
    selm = selm.astype(ml_dtypes.bfloat16)

    nc = _get_nc()
    in_maps = []
    for i in range(NCORES):
        s = i * VSH
        in_maps.append(
            {
                "w": np.ascontiguousarray(w[:, s : s + VSH, :]),
                "sv": np.ascontiguousarray(sv[:, s : s + VSH]),
                "oh": oh,
                "sel": selm,
                "centb": np.ascontiguousarray(centb[:, s : s + VSH]),
                "avec": avec,
            }
        )

    res = run_bass_kernel_spmd(nc, in_maps, core_ids=list(range(NCORES)))
    last_exec_time_ns = res.exec_time_ns
    last_results = res
    return np.concatenate([res.results[i]["out"] for i in range(NCORES)], axis=1)


# revision 31
# speedup vs baseline: 1.1583x; 1.1464x over previous
"""Trainium2 Bass kernel for AttributionCentroidTracker.

Reference computation (B=512, V=32768, C=16):
    Wg[b, v]   = W_eff[b, v, labels[b]]
    attr[b, v] = |sparse_vector[b, v] * Wg[b, v]|
    sums[c, v] = segment_sum(attr, labels)       # [C, V]
    mean       = sums / max(counts, 1)
    out[c]     = centroids[c]                     if counts[c] == 0
               = mean[c]                          if not initialized[c]
               = M*centroids[c] + (1-M)*mean[c]   otherwise

Device strategy (8 cores, sharded along V — per-class sums are complete
locally per V-slice, so no cross-core reduction is needed):
  - b (512) lives on the 128 SBUF partitions in 4 groups of 128.
  - W streams in as bf16 (SWDGE cast-DMA) [128, 4*VC*16] tiles.
  - |W| in place on VectorE via a sign-bit-clearing bitwise AND on an
    int32 view (2 bf16 per element, 2x mode) — ISA has no abs ALU op.
  - VectorE multiplies by |sv| broadcast along c (stride-0, 1x mode).
  - Segment-sum on TensorE with PLAIN one-hot lhsT [128,16] per batch
    group and CONTIGUOUS rhs [128,512] (v32*c16 natural layout), N=512
    matmuls accumulating over the 4 groups into 4 PSUM banks per tile.
    psum[c', (v,c)] holds per-class sums of ALL 16 channels; only the
    diagonal c'==c is wanted.
  - ScalarE evacuates each bank scaled by a_c into a bf16 staging tile;
    every TBATCH tiles the diagonal is pulled out with 16 accumulating
    selection matmuls E_cc^T @ stage[:, :, :, c] (stride-16 rhs) into a
    psum tile which VectorE adds into the f32 accumulator out_sb,
    pre-loaded with b_c*centroids (host-computed).
  - a/b host math: a = (init ? (1-M)/n : 1/n) if present else 0,
    b = (init ? M : 0) if present else 1.
"""

import os
import sys

import numpy as np

if "/opt/trn_rl_repo" not in sys.path:
    sys.path.insert(0, "/opt/trn_rl_repo")

B, V, C = 512, 32768, 16
NCORES = 8
VSH = V // NCORES            # 4096 columns of V per core
P = 128                      # SBUF partitions
BG = B // P                  # 4 batch groups
VC = 128                     # v-chunk per tile
NVC = VSH // VC              # 32 tiles per core
NSUB = 4                     # psum banks per tile (32 v each)
VSUB = VC // NSUB            # 32
TBATCH = 4                   # tiles per extraction batch
NQ = 4                       # sv quarter-loads
VQ = VSH // NQ               # 1024
STEPS_PER_EPOCH = 1000
MOMENTUM = 1.0 - 2.0 / (STEPS_PER_EPOCH + 1)

_CACHE = {}

last_exec_time_ns = None
last_results = None


def _build_nc():
    import concourse.bacc as bacc
    import concourse.tile as tile
    from concourse import mybir

    f32 = mybir.dt.float32
    bf16 = mybir.dt.bfloat16
    Copy = mybir.ActivationFunctionType.Copy
    nc = bacc.Bacc("TRN2", target_bir_lowering=False, debug=False)

    w = nc.dram_tensor("w", [B, VSH, C], f32, kind="ExternalInput")
    sv = nc.dram_tensor("sv", [B, VSH], f32, kind="ExternalInput")
    oh = nc.dram_tensor("oh", [P, BG * C], bf16, kind="ExternalInput")
    sel = nc.dram_tensor("sel", [C, C * C], bf16, kind="ExternalInput")
    centb = nc.dram_tensor("centb", [C, VSH], f32, kind="ExternalInput")
    avec = nc.dram_tensor("avec", [C, 1], f32, kind="ExternalInput")
    out = nc.dram_tensor("out", [C, VSH], f32, kind="ExternalOutput")

    # b = g*128 + p  ->  partition p, group g
    w_r = w.ap().rearrange("(g p) v c -> p g v c", p=P)      # [128, 4, VSH, 16]
    sv_r = sv.ap().rearrange("(g p) v -> p g v", p=P)        # [128, 4, VSH]

    with tile.TileContext(nc) as tc:
        with (
            tc.tile_pool(name="const", bufs=1) as cpool,
            tc.tile_pool(name="wp", bufs=6) as wpool,
            tc.tile_pool(name="stg", bufs=2) as spool,
            tc.tile_pool(name="psum", bufs=8, space="PSUM") as ppool,
        ):
            # |sv| as bf16, loaded in NQ v-quarters (quarter-major layout so
            # each quarter lands contiguously per partition) so the first
            # tile's multiply doesn't wait on the whole 8 MB transfer.
            svt = cpool.tile([P, BG * VSH], dtype=bf16)
            svt4 = svt[:].rearrange("p (q g v) -> p q g v", q=NQ, g=BG)

            def issue_sv_quarter(q):
                qsl = slice(q * VQ, (q + 1) * VQ)
                nc.gpsimd.dma_start(out=svt4[:, q], in_=sv_r[:, :, qsl])
                # abs via sign-bit clear on an int32 view (2 bf16 per elem)
                qv = svt4[:, q].bitcast(mybir.dt.int32)
                nc.vector.tensor_scalar(
                    out=qv,
                    in0=qv,
                    scalar1=0x7FFF7FFF,
                    scalar2=None,
                    op0=mybir.AluOpType.bitwise_and,
                )

            issue_sv_quarter(0)

            oh_sb = cpool.tile([P, BG * C], dtype=bf16)
            nc.sync.dma_start(out=oh_sb[:], in_=oh.ap())
            sel_sb = cpool.tile([C, C * C], dtype=bf16)
            nc.sync.dma_start(out=sel_sb[:], in_=sel.ap())
            avec_sb = cpool.tile([C, 1], dtype=f32)
            nc.sync.dma_start(out=avec_sb[:], in_=avec.ap())

            # accumulator pre-loaded with b_c * centroids
            out_sb = cpool.tile([C, VSH], dtype=f32)
            nc.sync.dma_start(out=out_sb[:], in_=centb.ap())

            def issue_w_dma(i):
                vlo = i * VC
                wt = wpool.tile([P, BG * VC * C], dtype=bf16, tag="wt")
                wt4 = wt[:].rearrange("p (g v c) -> p g v c", g=BG, v=VC)
                nc.gpsimd.dma_start(out=wt4, in_=w_r[:, :, vlo : vlo + VC, :])
                return wt, wt4

            PREFETCH = 3
            prefetched = {}
            for i in range(min(PREFETCH, NVC)):
                prefetched[i] = issue_w_dma(i)

            nsv = 1
            stage = None
            for i in range(NVC):
                vlo = i * VC
                ib = i % TBATCH

                if i + PREFETCH < NVC:
                    prefetched[i + PREFETCH] = issue_w_dma(i + PREFETCH)
                # keep sv quarters two tiles ahead of first use
                while nsv < NQ and i >= (nsv * NVC // NQ) - 2:
                    issue_sv_quarter(nsv)
                    nsv += 1

                wt, wt4 = prefetched.pop(i)

                # |W| in place: sign-bit clear on an int32 view (DVE 2x)
                wv = wt[:].bitcast(mybir.dt.int32)
                nc.vector.tensor_scalar(
                    out=wv,
                    in0=wv,
                    scalar1=0x7FFF7FFF,
                    scalar2=None,
                    op0=mybir.AluOpType.bitwise_and,
                )
                # Y = |W| * |sv|  (|sv| broadcast along c, DVE 1x)
                qi, vq = divmod(vlo, VQ)
                in1 = (
                    svt4[:, qi, :, vq : vq + VC]
                    .unsqueeze(3)
                    .broadcast_to([P, BG, VC, C])
                )
                nc.vector.tensor_tensor(
                    out=wt4, in0=wt4, in1=in1, op=mybir.AluOpType.mult
                )

                # segment-sum: ps[c', (v32, c)] += oh_g^T @ Y_g
                # (s outer / g inner so bank s finishes early and its
                # evacuation overlaps the remaining banks' matmuls)
                if ib == 0:
                    stage = spool.tile(
                        [C, TBATCH * VC * C], dtype=bf16, tag="stage"
                    )
                for s in range(NSUB):
                    ps = ppool.tile(
                        [C, VSUB * C],
                        dtype=mybir.dt.float32,
                        tag="ps",
                        name=f"ps{s}_{i}",
                    )
                    for g in range(BG):
                        off = g * (VC * C) + s * (VSUB * C)
                        nc.tensor.matmul(
                            out=ps[:],
                            lhsT=oh_sb[:, g * C : (g + 1) * C],
                            rhs=wt[:, off : off + VSUB * C],
                            start=(g == 0),
                            stop=(g == BG - 1),
                        )
                    # evacuate scaled by a_c into the bf16 staging tile
                    soff = (ib * NSUB + s) * (VSUB * C)
                    nc.scalar.activation(
                        stage[:, soff : soff + VSUB * C],
                        ps[:],
                        Copy,
                        bias=0.0,
                        scale=avec_sb[:],
                    )

                # extraction batch: diagonal (c', (v,c)) c'==c via 16
                # accumulating selection matmuls E_cc^T @ stage[:, :, :, c]
                if ib == TBATCH - 1:
                    nchunk = TBATCH * NSUB
                    ps2 = ppool.tile(
                        [C, TBATCH * VC],
                        dtype=mybir.dt.float32,
                        tag="ps",
                        name=f"ps_diag_{i}",
                    )
                    stg4 = stage[:].rearrange(
                        "q (k v c) -> q k v c", k=nchunk, v=VSUB
                    )
                    for c in range(C):
                        nc.tensor.matmul(
                            out=ps2[:],
                            lhsT=sel_sb[:, c * C : (c + 1) * C],
                            rhs=stg4[:, :, :, c],
                            start=(c == 0),
                            stop=(c == C - 1),
                        )
                    ooff = (i - (TBATCH - 1)) * VC
                    nc.vector.tensor_tensor(
                        out=out_sb[:, ooff : ooff + TBATCH * VC],
                        in0=out_sb[:, ooff : ooff + TBATCH * VC],
                        in1=ps2[:],
                        op=mybir.AluOpType.add,
                    )

            nc.sync.dma_start(out=out.ap(), in_=out_sb[:])

    nc.finalize()
    return nc


def _get_nc():
    if "nc" not in _CACHE:
        _CACHE["nc"] = _build_nc()
    return _CACHE["nc"]


def kernel(sparse_vector, W_eff, labels, centroids, initialized):
    global last_exec_time_ns, last_results
    import ml_dtypes
    from concourse.bass_utils import run_bass_kernel_spmd

    sv = np.ascontiguousarray(np.asarray(sparse_vector, dtype=np.float32))
    w = np.asarray(W_eff, dtype=np.float32)
    lab = np.asarray(labels).astype(np.int64)
    cent = np.asarray(centroids, dtype=np.float32)
    init = np.asarray(initialized).astype(bool)

    # Host-side label-derived constants (tiny) — keep the program generic.
    ohm = lab[:, None] == np.arange(C)[None, :]          # [B, C] bool
    counts = ohm.sum(axis=0).astype(np.float64)          # [C]
    present = counts > 0
    safe = np.maximum(counts, 1.0)
    a = np.where(present, np.where(init, (1.0 - MOMENTUM) / safe, 1.0 / safe), 0.0)
    b = np.where(present, np.where(init, MOMENTUM, 0.0), 1.0)
    avec = a.astype(np.float32).reshape(C, 1)
    centb = (b[:, None] * cent.astype(np.float64)).astype(np.float32)  # [C, V]

    # Plain one-hot lhsT blocks: oh[p, g*C + c] = 1 iff labels[g*128+p]==c
    lab2 = lab.reshape(BG, P)                            # [g, p]
    oh = np.zeros((P, BG * C), np.float32)
    for g in range(BG):
        oh[np.arange(P), g * C + lab2[g]] = 1.0
    oh = oh.astype(ml_dtypes.bfloat16)

    # Diagonal-selection lhsT blocks: sel[p, c*C+m] = 1 iff p==c==m
    selm = np.zeros((C, C * C), np.float32)
    for c in range(C):
        selm[c, c * C + c] = 1.0
    selm = selm.astype(ml_dtypes.bfloat16)

    nc = _get_nc()
    in_maps = []
    for i in range(NCORES):
        s = i * VSH
        in_maps.append(
            {
                "w": np.ascontiguousarray(w[:, s : s + VSH, :]),
                "sv": np.ascontiguousarray(sv[:, s : s + VSH]),
                "oh": oh,
                "sel": selm,
                "centb": np.ascontiguousarray(centb[:, s : s + VSH]),
                "avec": avec,
            }
        )

    res = run_bass_kernel_spmd(nc, in_maps, core_ids=list(range(NCORES)))
    last_exec_time_ns = res.exec_time_ns
    last_results = res
    return np.concatenate([res.results[i]["out"] for i in range(NCORES)], axis=1)
